# revision 34
# baseline (speedup 1.0000x reference)
"""Trainium2 Bass kernel for nn_AttentionBlock (B=4, C=64, H=W=64, INTER=8).

Sharding: 8 cores = 4 batches x 2 query-halves. Each core computes, for its
batch b and its half of the query pixels (n), the full attention output
gamma * (V @ softmax(Q^T K)^T) + x over all m=4096 keys.

SPMD uniformity trick: the host permutes each core's pixel columns so that
columns [0, 2048) are the core's OWN query half and [2048, 4096) are the
other half. Attention is permutation-invariant over keys, so every core runs
the identical program on differently-permuted data.

v2 design (ACT-exp is the fundamental bottleneck at ~55us busy/core; PE is
made non-critical so HAM throttling to 1.2 GHz cannot dominate):
  1. kq setup: wqk4 [65, 128] holds 4 replicated column-groups of
     [Wk.T|0..|Wq.T] (+bias row), so ONE matmul per 512-pixel chunk yields
     k and q already replicated in partition groups 32i. Small DVE copies
     distribute k m-blocks to row-group-aligned k4 and q chunks to q4.
  2. Energy matmuls are 4-way ROW-TILED (tile_position=(32i,0), K=8 of 32):
     4 concurrent m-block matmuls in the PE array -> ~3x energy throughput.
  3. exp groups of GRP=3 m-blocks (3 PSUM banks, 2 bufs): 44 ACT
     instructions instead of 64 amortizes the ~470ns/instr ACT overhead.
  4. AV (vT_aug^T @ expE) accumulates out_aug[65, 512] per chunk; row 64
     (vT ones column) is the softmax denominator.
  5. gamma*bv is folded into the residual input on the host (out =
     gamma*(V@A/d) + gamma*bv + x, since sum_m A[n,m] = d[n]), so Wv has
     no bias row and vt matmuls contract over K=64.
  6. The natural_log_exp_and_others ACT table set (ln+exp) is preloaded
     once, so the last-chunk 1/x = exp(-ln(x)) trick causes no table
     reloads. Mid-stream chunks use DVE reciprocal (slow but off the
     critical engine).

No max-subtraction is needed in softmax: |energy| <~ 15 for this problem's
fixed input distribution, well within fp32 exp range.
"""

import os
import sys
import types
import numpy as np
import ml_dtypes


def _ensure_ntff_hook_importable():
    """bass_utils imports antenv.axon_hooks when tracing is requested via
    BASS_TRACE; some images lack that module. Provide it (backed by the
    ctypes hook from trn_boot when available, else a None hook, which
    bass_utils handles by skipping the trace)."""
    try:
        import antenv.axon_hooks  # noqa: F401
        return
    except ImportError:
        pass
    hook = None
    try:
        from trn_agent_boot.trn_boot import _ntff_profile_via_ctypes
        so = "/opt/axon/libaxon_pjrt.so"
        if os.path.exists(so):
            hook = _ntff_profile_via_ctypes(so)
    except Exception:
        hook = None
    mod = types.ModuleType("antenv.axon_hooks")
    mod.get_axon_ntff_profile_hook = lambda: hook
    sys.modules["antenv.axon_hooks"] = mod

B, C, H, W = 4, 64, 64, 64
N = H * W              # 4096 pixels
NHALF = N // 2         # 2048 query pixels per core
INTER = C // 8         # 8
NCORES = 8
MBLK = 128             # m-block (PSUM partition tile)
NCHUNK = 512           # query-chunk (PSUM bank free size)
NJ = N // MBLK         # 32 m-blocks
NT = NHALF // NCHUNK   # 4 query chunks
GRP = int(os.environ.get("KGRP", "3"))      # m-blocks per exp instruction
NGRP = (NJ + GRP - 1) // GRP                # exp groups per chunk
ROWTILE = int(os.environ.get("KROWTILE", "1"))  # 4-way PE row tiling
KFILL = int(os.environ.get("KFILL", "0"))   # HAM-warming filler matmuls/group
ACT_SET_LN_EXP = 6     # act_info.json index of natural_log_exp_and_others

_compiled = {}
LAST_RESULT = None


def _grp_of(j):
    return min(j // GRP, NGRP - 1)


def _build():
    import concourse.bacc as bacc
    import concourse.mybir as mybir
    from concourse.tile import TileContext

    dt = mybir.dt
    f32, bf16 = dt.float32, dt.bfloat16
    EXP = mybir.ActivationFunctionType.Exp
    LN = mybir.ActivationFunctionType.Ln

    nc = bacc.Bacc("TRN2", target_bir_lowering=False, debug=False,
                   num_devices=NCORES)

    # host-prepped inputs (see kernel() below)
    xbh = nc.dram_tensor("xbh", [130, NHALF], bf16, kind="ExternalInput").ap()
    xres = nc.dram_tensor("xres", [C, NHALF], f32, kind="ExternalInput").ap()
    wqk = nc.dram_tensor("wqk", [C + 1, 16], bf16, kind="ExternalInput").ap()
    wv = nc.dram_tensor("wv_", [C, C], bf16, kind="ExternalInput").ap()
    out = nc.dram_tensor("out", [C, NHALF], f32, kind="ExternalOutput").ap()

    with TileContext(nc) as tc:
        with tc.tile_pool(name="const", bufs=1) as cp, \
             tc.tile_pool(name="eps", bufs=2, space="PSUM") as eps, \
             tc.tile_pool(name="ops", bufs=8 - 2 * (GRP + 1),
                          space="PSUM") as ops, \
             tc.tile_pool(name="work", bufs=3) as wp, \
             tc.tile_pool(name="fin", bufs=2) as fp:

            # Preload the ln+exp activation-table set so neither the exp
            # stream nor the tail 1/x = exp(-ln x) forces a table switch.
            nc.scalar.add_instruction(mybir.InstLoadActFuncSet(
                name=nc.scalar.bass.get_next_instruction_name(),
                act_func_set_id=ACT_SET_LN_EXP, ins=[], outs=[]))

            # DMA issue order matters: the first kq matmul needs xqo piece
            # one + wqk; wv is needed by the first vt group shortly after;
            # xres only by the chunk-0 epilogue.
            wqk_t = cp.tile([C + 1, 16], bf16, tag="wqk", name="wqk_t")
            nc.sync.dma_start(out=wqk_t[:, :], in_=wqk)
            xqo = cp.tile([C + 1, NHALF], bf16, tag="xqo", name="xqo")
            nc.sync.dma_start(out=xqo[:, 0:NCHUNK], in_=xbh[0:C + 1, 0:NCHUNK])
            wv_t = cp.tile([C, C], bf16, tag="wv", name="wv_t")
            nc.sync.dma_start(out=wv_t[:, :], in_=wv)
            nc.sync.dma_start(out=xqo[:, NCHUNK:], in_=xbh[0:C + 1, NCHUNK:])
            xqt = cp.tile([C + 1, NHALF], bf16, tag="xqt", name="xqt")
            nc.sync.dma_start(out=xqt[:, :], in_=xbh[C + 1:2 * C + 2, :])
            xr_t = cp.tile([C, NHALF], f32, tag="xr", name="xr_t")
            nc.sync.dma_start(out=xr_t[:, :], in_=xres)
            ones64 = cp.tile([1, C], bf16, tag="ones64", name="ones64")
            nc.vector.memset(ones64[:, :], 1.0)

            # kall/qall: k (q) of every 512-pixel chunk replicated across
            # partition groups {0,32,64,96}+0..7 by COLUMN-TILED kq matmuls
            # (4 concurrent tiles, tile_position=(0,32i)), so a single fat
            # PSUM->SBUF cast replaces the whole per-group copy chain.
            # Energy lhsT for m-block j: kall[32(j%4)+0..8,
            # 512(j//4)+128(j%4) ..+128]; rhs: qall[32i+0..8, chunk].
            kall = cp.tile([128, N], bf16, tag="kall", name="kall")
            qall = cp.tile([128, NHALF], bf16, tag="qall", name="qall")
            vt = cp.tile([128, NJ * (C + 1)], bf16, tag="vt", name="vt")
            vt3 = vt.rearrange("p (j c) -> p j c", c=C + 1)
            nc.vector.memset(vt3[:, :, C], 1.0)

            def emit_kq(kc):
                """kq chunk kc (0-3 own half: k chunk kc + q chunk kc;
                4-7 other half: k chunk kc only)."""
                own = kc < NT
                srct = xqo if own else xqt
                t = kc % NT
                rhs = srct[:, NCHUNK * t:NCHUNK * (t + 1)]
                kp = ops.tile([128, NCHUNK], f32, tag="o", name="kp")
                for i in range(4):
                    nc.tensor.matmul(kp[32 * i:32 * i + INTER, :],
                                     wqk_t[:, 0:INTER], rhs,
                                     start=True, stop=True,
                                     tile_position=(0, 32 * i))
                nc.vector.tensor_copy(
                    kall[:, NCHUNK * kc:NCHUNK * (kc + 1)], kp[:, :])
                if own:
                    qp = ops.tile([128, NCHUNK], f32, tag="o", name="qp")
                    for i in range(4):
                        nc.tensor.matmul(qp[32 * i:32 * i + INTER, :],
                                         wqk_t[:, INTER:2 * INTER], rhs,
                                         start=True, stop=True,
                                         tile_position=(0, 32 * i))
                    nc.vector.tensor_copy(
                        qall[:, NCHUNK * t:NCHUNK * (t + 1)], qp[:, :])

            def emit_vt(g8):
                """vt group g8: m-blocks 8*g8 .. 8*g8+7."""
                v_p = ops.tile([128, 8 * C], f32, tag="o", name="v_p")
                for jj in range(8):
                    jl = 8 * g8 + jj
                    srct = xqo if jl < NJ // 2 else xqt
                    blk = (jl % (NJ // 2)) * MBLK
                    nc.tensor.matmul(
                        v_p[:, C * jj:C * (jj + 1)],
                        srct[0:C, blk:blk + MBLK],
                        wv_t[:, :], start=True, stop=True)
                v_p8 = v_p.rearrange("p (j c) -> p j c", c=C)
                nc.vector.tensor_copy(vt3[:, 8 * g8:8 * g8 + 8, 0:C], v_p8)

            # e-tiles keyed by GLOBAL group index so HAM-warming fillers can
            # pre-touch the next group's slot.
            e_tiles = {}

            def get_e(gg):
                if gg not in e_tiles:
                    e_tiles[gg] = eps.tile([128, NCHUNK * GRP], f32,
                                           tag="e", name="e")
                return e_tiles[gg]

            def energy_mm(t, g, j, e, dup=False):
                sl = slice(NCHUNK * (j - GRP * g), NCHUNK * (j - GRP * g + 1))
                i = j % 4 if ROWTILE else 0
                w = NCHUNK * (j // 4) + MBLK * (j % 4)
                nc.tensor.matmul(
                    e[:, sl],
                    kall[32 * i:32 * i + INTER, w:w + MBLK],
                    qall[32 * i:32 * i + INTER, NCHUNK * t:NCHUNK * (t + 1)],
                    start=True, stop=True,
                    tile_position=(32 * i, 0) if ROWTILE else None)

            def emit_exp(t, g):
                gg = t * NGRP + g
                j0, j1 = GRP * g, min(GRP * (g + 1), NJ)
                e = get_e(gg)
                ex = wp.tile([128, NCHUNK * GRP], bf16, tag="ex", name="ex")
                nc.scalar.activation(ex[:, 0:NCHUNK * (j1 - j0)],
                                     e[:, 0:NCHUNK * (j1 - j0)], EXP)
                return ex

            def emit_av(oa, ex, g):
                j0, j1 = GRP * g, min(GRP * (g + 1), NJ)
                for j in range(j0, j1):
                    nc.tensor.matmul(oa[:, :], vt3[:, j, :],
                                     ex[:, NCHUNK * (j - j0):
                                        NCHUNK * (j - j0 + 1)],
                                     start=(j == 0), stop=(j == NJ - 1))

            def emit_epilogue(t, oa):
                # normalize + residual + store (PE-free, pipelined halves)
                nparts = 2
                HC = NCHUNK // nparts
                if t < NT - 1:
                    # copy-out first: frees oa's PSUM slot immediately so
                    # the next chunk's oa allocates without stalling, and
                    # later DVE ops read SBUF (faster access) instead.
                    oas = fp.tile([C + 1, NCHUNK], f32, tag="oas", name="oas")
                    nc.vector.tensor_copy(oas[:, :], oa[:, :])
                    src = oas
                    for hh in range(nparts):
                        hs = slice(HC * hh, HC * (hh + 1))
                        gs = slice(NCHUNK * t + HC * hh,
                                   NCHUNK * t + HC * (hh + 1))
                        rec = fp.tile([1, HC], f32, tag=f"rec{hh}", name="rec")
                        nc.vector.reciprocal(rec[:, :], src[C:C + 1, hs])
                        bcs = fp.tile([C, HC], f32, tag=f"bcs{hh}", name="bcs")
                        nc.gpsimd.partition_broadcast(bcs[:, :], rec[:, :])
                        t1 = fp.tile([C, HC], f32, tag=f"t1{hh}", name="t1")
                        nc.vector.tensor_mul(t1[:, :], src[0:C, hs], bcs[:, :])
                        fin = fp.tile([C, HC], f32, tag=f"fin{hh}", name="fin")
                        nc.vector.tensor_add(fin[:, :], t1[:, :], xr_t[:, gs])
                        nc.sync.dma_start(out=out[:, gs], in_=fin[:, :])
                else:
                    # latency-critical tail: per-half 1/x via ACT ln+exp
                    # (table set already resident; no reloads). oa is
                    # copied to SBUF once (DVE can read only one PSUM
                    # operand); the reciprocal-row broadcast runs on the
                    # idle PE (ones64^T @ rec -> PSUM) instead of the
                    # serial gpsimd queue. Phase-ordered so the ACT chain
                    # finishes first and the halves pipeline.
                    oas = fp.tile([C + 1, NCHUNK], f32, tag="oas",
                                  name="oas")
                    nc.vector.tensor_copy(oas[:, :], oa[:, :])
                    recs = []
                    for hh in range(nparts):
                        hs = slice(HC * hh, HC * (hh + 1))
                        lnt = fp.tile([1, HC], f32, tag=f"lnt{hh}", name="lnt")
                        nc.scalar.activation(lnt[:, :], oas[C:C + 1, hs],
                                             mybir.ActivationFunctionType.Ln)
                        rec = fp.tile([1, HC], bf16, tag=f"recf{hh}",
                                      name="recf")
                        nc.scalar.activation(rec[:, :], lnt[:, :], EXP,
                                             scale=-1.0)
                        recs.append(rec)
                    bc = ops.tile([C, NCHUNK], f32, tag="o", name="bc")
                    for hh in range(nparts):
                        hs = slice(HC * hh, HC * (hh + 1))
                        nc.tensor.matmul(bc[:, hs], ones64[:, :],
                                         recs[hh][:, :],
                                         start=True, stop=True)
                    for hh in range(nparts):
                        hs = slice(HC * hh, HC * (hh + 1))
                        gs = slice(NCHUNK * t + HC * hh,
                                   NCHUNK * t + HC * (hh + 1))
                        t1 = fp.tile([C, HC], f32, tag=f"t1{hh}", name="t1")
                        nc.vector.tensor_mul(t1[:, :], oas[0:C, hs],
                                             bc[:, hs])
                        fin = fp.tile([C, HC], f32, tag=f"fin{hh}", name="fin")
                        nc.vector.tensor_add(fin[:, :], t1[:, :], xr_t[:, gs])
                        nc.sync.dma_start(out=out[:, gs], in_=fin[:, :])

            # ---- emission: energies per exp-group (3 m-blocks, distinct
            # row groups -> concurrent burst); exp(g) after its energies;
            # AV lags one group so the PE can run ahead of the ACT.
            kq_done = 0
            vt_done = 0

            for t in range(NT):
                oa = ops.tile([C + 1, NCHUNK], f32, tag="o", name="oa")
                exs = {}
                for g in range(NGRP):
                    jlast = min(GRP * (g + 1), NJ) - 1
                    if t == 0:
                        while kq_done <= min(jlast // 4, 2 * NT - 1):
                            emit_kq(kq_done)
                            kq_done += 1
                    for j in range(GRP * g, jlast + 1):
                        energy_mm(t, g, j, get_e(t * NGRP + g))
                    exs[g] = emit_exp(t, g)
                    if g >= 1:
                        if t == 0:
                            jprev = min(GRP * g, NJ) - 1
                            while vt_done <= min(jprev // 8, NJ // 8 - 1):
                                emit_vt(vt_done)
                                vt_done += 1
                        emit_av(oa, exs.pop(g - 1), g - 1)
                if t == 0:
                    while vt_done < NJ // 8:
                        emit_vt(vt_done)
                        vt_done += 1
                emit_av(oa, exs.pop(NGRP - 1), NGRP - 1)
                emit_epilogue(t, oa)

    nc.compile()
    return nc


def _get_compiled():
    if "nc" not in _compiled:
        _compiled["nc"] = _build()
    return _compiled["nc"]


def kernel(x, Wq, bq, Wk, bk, Wv, bv, gamma):
    global LAST_RESULT
    _ensure_ntff_hook_importable()
    from concourse.bass_utils import run_bass_kernel_spmd

    nc = _get_compiled()

    x = np.asarray(x, dtype=np.float32)
    xf = x.reshape(B, C, N)
    Wq, Wk, Wv = np.asarray(Wq), np.asarray(Wk), np.asarray(Wv)
    bq, bk, bv = np.asarray(bq), np.asarray(bk), np.asarray(bv)
    gval = float(np.asarray(gamma).reshape(-1)[0])

    # wqk [65, 16]: k weights at cols 0-7, q at cols 8-15, bias row at 64.
    wqk_a = np.zeros((C + 1, 16), np.float32)
    wqk_a[0:C, 0:INTER] = Wk.T
    wqk_a[C, 0:INTER] = bk
    wqk_a[0:C, INTER:2 * INTER] = Wq.T
    wqk_a[C, INTER:2 * INTER] = bq
    wqk_a = wqk_a.astype(ml_dtypes.bfloat16)
    wv_a = np.ascontiguousarray(gval * Wv.T).astype(ml_dtypes.bfloat16)

    in_maps = []
    for core in range(NCORES):
        b, h = divmod(core, 2)
        own = xf[b][:, h * NHALF:(h + 1) * NHALF]
        oth = xf[b][:, (1 - h) * NHALF:(2 - h) * NHALF]
        ones = np.ones((1, NHALF), dtype=np.float32)
        xbh_core = np.concatenate([own, ones, oth, ones],
                                  axis=0).astype(ml_dtypes.bfloat16)
        # gamma*bv folded into the residual (sum_m A[n,m] = denom[n])
        xres_core = own + gval * bv[:, None]
        in_maps.append({
            "xbh": np.ascontiguousarray(xbh_core),
            "xres": np.ascontiguousarray(xres_core, dtype=np.float32),
            "wqk": wqk_a, "wv_": wv_a,
        })

    trace = bool(os.environ.get("KTRACE"))
    res = run_bass_kernel_spmd(nc, in_maps, list(range(NCORES)), trace=trace)
    LAST_RESULT = res

    outf = np.empty((B, C, N), dtype=np.float32)
    for core in range(NCORES):
        b, h = divmod(core, 2)
        outf[b][:, h * NHALF:(h + 1) * NHALF] = res.results[core]["out"]
    return outf.reshape(B, C, H, W)


# revision 35
# speedup vs baseline: 1.1353x; 1.1353x over previous
"""Trainium2 Bass kernel for nn_AttentionBlock (B=4, C=64, H=W=64, INTER=8).

Sharding: 8 cores = 4 batches x 2 query-halves. Each core computes, for its
batch b and its half of the query pixels (n), the full attention output
gamma * (V @ softmax(Q^T K)^T) + x over all m=4096 keys.

SPMD uniformity trick: the host permutes each core's pixel columns so that
columns [0, 2048) are the core's OWN query half and [2048, 4096) are the
other half. Attention is permutation-invariant over keys, so every core runs
the identical program on differently-permuted data.

v2 design (ACT-exp is the fundamental bottleneck at ~55us busy/core; PE is
made non-critical so HAM throttling to 1.2 GHz cannot dominate):
  1. kq setup: wqk4 [65, 128] holds 4 replicated column-groups of
     [Wk.T|0..|Wq.T] (+bias row), so ONE matmul per 512-pixel chunk yields
     k and q already replicated in partition groups 32i. Small DVE copies
     distribute k m-blocks to row-group-aligned k4 and q chunks to q4.
  2. Energy matmuls are 4-way ROW-TILED (tile_position=(32i,0), K=8 of 32):
     4 concurrent m-block matmuls in the PE array -> ~3x energy throughput.
  3. exp groups of GRP=3 m-blocks (3 PSUM banks, 2 bufs): 44 ACT
     instructions instead of 64 amortizes the ~470ns/instr ACT overhead.
  4. AV (vT_aug^T @ expE) accumulates out_aug[65, 512] per chunk; row 64
     (vT ones column) is the softmax denominator.
  5. gamma*bv is folded into the residual input on the host (out =
     gamma*(V@A/d) + gamma*bv + x, since sum_m A[n,m] = d[n]), so Wv has
     no bias row and vt matmuls contract over K=64.
  6. The natural_log_exp_and_others ACT table set (ln+exp) is preloaded
     once, so the last-chunk 1/x = exp(-ln(x)) trick causes no table
     reloads. Mid-stream chunks use DVE reciprocal (slow but off the
     critical engine).

No max-subtraction is needed in softmax: |energy| <~ 15 for this problem's
fixed input distribution, well within fp32 exp range.
"""

import os
import sys
import types
import numpy as np
import ml_dtypes


def _ensure_ntff_hook_importable():
    """bass_utils imports antenv.axon_hooks when tracing is requested via
    BASS_TRACE; some images lack that module. Provide it (backed by the
    ctypes hook from trn_boot when available, else a None hook, which
    bass_utils handles by skipping the trace)."""
    try:
        import antenv.axon_hooks  # noqa: F401
        return
    except ImportError:
        pass
    hook = None
    try:
        from trn_agent_boot.trn_boot import _ntff_profile_via_ctypes
        so = "/opt/axon/libaxon_pjrt.so"
        if os.path.exists(so):
            hook = _ntff_profile_via_ctypes(so)
    except Exception:
        hook = None
    mod = types.ModuleType("antenv.axon_hooks")
    mod.get_axon_ntff_profile_hook = lambda: hook
    sys.modules["antenv.axon_hooks"] = mod

B, C, H, W = 4, 64, 64, 64
N = H * W              # 4096 pixels
NHALF = N // 2         # 2048 query pixels per core
INTER = C // 8         # 8
NCORES = 8
MBLK = 128             # m-block (PSUM partition tile)
NCHUNK = 512           # query-chunk (PSUM bank free size)
NJ = N // MBLK         # 32 m-blocks
NT = NHALF // NCHUNK   # 4 query chunks
GRP = int(os.environ.get("KGRP", "3"))      # m-blocks per exp instruction
NGRP = (NJ + GRP - 1) // GRP                # exp groups per chunk
ROWTILE = int(os.environ.get("KROWTILE", "1"))  # 4-way PE row tiling
KFILL = int(os.environ.get("KFILL", "0"))   # HAM-warming filler matmuls/group
ACT_SET_LN_EXP = 6     # act_info.json index of natural_log_exp_and_others

_compiled = {}
LAST_RESULT = None


def _grp_of(j):
    return min(j // GRP, NGRP - 1)


def _build():
    import concourse.bacc as bacc
    import concourse.mybir as mybir
    from concourse.tile import TileContext

    dt = mybir.dt
    f32, bf16 = dt.float32, dt.bfloat16
    EXP = mybir.ActivationFunctionType.Exp
    LN = mybir.ActivationFunctionType.Ln

    nc = bacc.Bacc("TRN2", target_bir_lowering=False, debug=False,
                   num_devices=NCORES)

    # host-prepped inputs (see kernel() below)
    xbh = nc.dram_tensor("xbh", [130, NHALF], bf16, kind="ExternalInput").ap()
    xres = nc.dram_tensor("xres", [C, NHALF], f32, kind="ExternalInput").ap()
    wqk = nc.dram_tensor("wqk", [C + 1, 16], bf16, kind="ExternalInput").ap()
    wv = nc.dram_tensor("wv_", [C, C], bf16, kind="ExternalInput").ap()
    out = nc.dram_tensor("out", [C, NHALF], f32, kind="ExternalOutput").ap()

    with TileContext(nc) as tc:
        with tc.tile_pool(name="const", bufs=1) as cp, \
             tc.tile_pool(name="eps", bufs=2, space="PSUM") as eps, \
             tc.tile_pool(name="ops", bufs=8 - 2 * GRP,
                          space="PSUM") as ops, \
             tc.tile_pool(name="work", bufs=3) as wp, \
             tc.tile_pool(name="fin", bufs=2) as fp:

            # Preload the ln+exp activation-table set so neither the exp
            # stream nor the tail 1/x = exp(-ln x) forces a table switch.
            nc.scalar.add_instruction(mybir.InstLoadActFuncSet(
                name=nc.scalar.bass.get_next_instruction_name(),
                act_func_set_id=ACT_SET_LN_EXP, ins=[], outs=[]))

            # DMA issue order matters: the first kq matmul needs xqo piece
            # one + wqk; wv is needed by the first vt group shortly after;
            # xres only by the chunk-0 epilogue.
            wqk_t = cp.tile([C + 1, 16], bf16, tag="wqk", name="wqk_t")
            nc.sync.dma_start(out=wqk_t[:, :], in_=wqk)
            xqo = cp.tile([C + 1, NHALF], bf16, tag="xqo", name="xqo")
            nc.sync.dma_start(out=xqo[:, 0:NCHUNK], in_=xbh[0:C + 1, 0:NCHUNK])
            wv_t = cp.tile([C, C], bf16, tag="wv", name="wv_t")
            nc.sync.dma_start(out=wv_t[:, :], in_=wv)
            nc.sync.dma_start(out=xqo[:, NCHUNK:], in_=xbh[0:C + 1, NCHUNK:])
            xqt = cp.tile([C + 1, NHALF], bf16, tag="xqt", name="xqt")
            nc.sync.dma_start(out=xqt[:, :], in_=xbh[C + 1:2 * C + 2, :])
            xr_t = cp.tile([C, NHALF], f32, tag="xr", name="xr_t")
            nc.sync.dma_start(out=xr_t[:, :], in_=xres)
            ones64 = cp.tile([1, C], bf16, tag="ones64", name="ones64")
            nc.vector.memset(ones64[:, :], 1.0)

            # kall/qall: k (q) of every 512-pixel chunk replicated across
            # partition groups {0,32,64,96}+0..7 by COLUMN-TILED kq matmuls
            # (4 concurrent tiles, tile_position=(0,32i)), so a single fat
            # PSUM->SBUF cast replaces the whole per-group copy chain.
            # Energy lhsT for m-block j: kall[32(j%4)+0..8,
            # 512(j//4)+128(j%4) ..+128]; rhs: qall[32i+0..8, chunk].
            kall = cp.tile([128, N], bf16, tag="kall", name="kall")
            qall = cp.tile([128, NHALF], bf16, tag="qall", name="qall")
            vt = cp.tile([128, NJ * (C + 1)], bf16, tag="vt", name="vt")
            vt3 = vt.rearrange("p (j c) -> p j c", c=C + 1)
            nc.vector.memset(vt3[:, :, C], 1.0)

            def emit_kq(kc):
                """kq chunk kc (0-3 own half: k chunk kc + q chunk kc;
                4-7 other half: k chunk kc only)."""
                own = kc < NT
                srct = xqo if own else xqt
                t = kc % NT
                rhs = srct[:, NCHUNK * t:NCHUNK * (t + 1)]
                kp = ops.tile([128, NCHUNK], f32, tag="o", name="kp")
                for i in range(4):
                    nc.tensor.matmul(kp[32 * i:32 * i + INTER, :],
                                     wqk_t[:, 0:INTER], rhs,
                                     start=True, stop=True,
                                     tile_position=(0, 32 * i))
                nc.vector.tensor_copy(
                    kall[:, NCHUNK * kc:NCHUNK * (kc + 1)], kp[:, :])
                if own:
                    qp = ops.tile([128, NCHUNK], f32, tag="o", name="qp")
                    for i in range(4):
                        nc.tensor.matmul(qp[32 * i:32 * i + INTER, :],
                                         wqk_t[:, INTER:2 * INTER], rhs,
                                         start=True, stop=True,
                                         tile_position=(0, 32 * i))
                    nc.vector.tensor_copy(
                        qall[:, NCHUNK * t:NCHUNK * (t + 1)], qp[:, :])

            def emit_vt(g8):
                """vt group g8: m-blocks 8*g8 .. 8*g8+7."""
                v_p = ops.tile([128, 8 * C], f32, tag="o", name="v_p")
                for jj in range(8):
                    jl = 8 * g8 + jj
                    srct = xqo if jl < NJ // 2 else xqt
                    blk = (jl % (NJ // 2)) * MBLK
                    nc.tensor.matmul(
                        v_p[:, C * jj:C * (jj + 1)],
                        srct[0:C, blk:blk + MBLK],
                        wv_t[:, :], start=True, stop=True)
                v_p8 = v_p.rearrange("p (j c) -> p j c", c=C)
                nc.vector.tensor_copy(vt3[:, 8 * g8:8 * g8 + 8, 0:C], v_p8)

            # e-tiles keyed by GLOBAL group index so HAM-warming fillers can
            # pre-touch the next group's slot.
            e_tiles = {}

            def get_e(gg):
                if gg not in e_tiles:
                    e_tiles[gg] = eps.tile([128, NCHUNK * GRP], f32,
                                           tag="e", name="e")
                return e_tiles[gg]

            def energy_mm(t, g, j, e, dup=False):
                sl = slice(NCHUNK * (j - GRP * g), NCHUNK * (j - GRP * g + 1))
                i = j % 4 if ROWTILE else 0
                w = NCHUNK * (j // 4) + MBLK * (j % 4)
                nc.tensor.matmul(
                    e[:, sl],
                    kall[32 * i:32 * i + INTER, w:w + MBLK],
                    qall[32 * i:32 * i + INTER, NCHUNK * t:NCHUNK * (t + 1)],
                    start=True, stop=True,
                    tile_position=(32 * i, 0) if ROWTILE else None)

            def emit_exp(t, g):
                gg = t * NGRP + g
                j0, j1 = GRP * g, min(GRP * (g + 1), NJ)
                e = get_e(gg)
                ex = wp.tile([128, NCHUNK * GRP], bf16, tag="ex", name="ex")
                nc.scalar.activation(ex[:, 0:NCHUNK * (j1 - j0)],
                                     e[:, 0:NCHUNK * (j1 - j0)], EXP)
                return ex

            def emit_av(oa, ex, g):
                j0, j1 = GRP * g, min(GRP * (g + 1), NJ)
                for j in range(j0, j1):
                    nc.tensor.matmul(oa[:, :], vt3[:, j, :],
                                     ex[:, NCHUNK * (j - j0):
                                        NCHUNK * (j - j0 + 1)],
                                     start=(j == 0), stop=(j == NJ - 1))

            def emit_epilogue(t, oa):
                # normalize + residual + store (PE-free, pipelined halves)
                nparts = 2
                HC = NCHUNK // nparts
                if t < NT - 1:
                    # copy-out first: frees oa's PSUM slot immediately so
                    # the next chunk's oa allocates without stalling, and
                    # later DVE ops read SBUF (faster access) instead.
                    oas = fp.tile([C + 1, NCHUNK], f32, tag="oas", name="oas")
                    nc.vector.tensor_copy(oas[:, :], oa[:, :])
                    src = oas
                    for hh in range(nparts):
                        hs = slice(HC * hh, HC * (hh + 1))
                        gs = slice(NCHUNK * t + HC * hh,
                                   NCHUNK * t + HC * (hh + 1))
                        rec = fp.tile([1, HC], f32, tag=f"rec{hh}", name="rec")
                        nc.vector.reciprocal(rec[:, :], src[C:C + 1, hs])
                        bcs = fp.tile([C, HC], f32, tag=f"bcs{hh}", name="bcs")
                        nc.gpsimd.partition_broadcast(bcs[:, :], rec[:, :])
                        t1 = fp.tile([C, HC], f32, tag=f"t1{hh}", name="t1")
                        nc.vector.tensor_mul(t1[:, :], src[0:C, hs], bcs[:, :])
                        fin = fp.tile([C, HC], f32, tag=f"fin{hh}", name="fin")
                        nc.vector.tensor_add(fin[:, :], t1[:, :], xr_t[:, gs])
                        nc.sync.dma_start(out=out[:, gs], in_=fin[:, :])
                else:
                    # latency-critical tail: per-half 1/x via ACT ln+exp
                    # (table set already resident; no reloads). oa is
                    # copied to SBUF once (DVE can read only one PSUM
                    # operand); the reciprocal-row broadcast runs on the
                    # idle PE (ones64^T @ rec -> PSUM) instead of the
                    # serial gpsimd queue. Phase-ordered so the ACT chain
                    # finishes first and the halves pipeline.
                    oas = fp.tile([C + 1, NCHUNK], f32, tag="oas",
                                  name="oas")
                    nc.vector.tensor_copy(oas[:, :], oa[:, :])
                    recs = []
                    for hh in range(nparts):
                        hs = slice(HC * hh, HC * (hh + 1))
                        lnt = fp.tile([1, HC], f32, tag=f"lnt{hh}", name="lnt")
                        nc.scalar.activation(lnt[:, :], oas[C:C + 1, hs],
                                             mybir.ActivationFunctionType.Ln)
                        rec = fp.tile([1, HC], bf16, tag=f"recf{hh}",
                                      name="recf")
                        nc.scalar.activation(rec[:, :], lnt[:, :], EXP,
                                             scale=-1.0)
                        recs.append(rec)
                    bc = ops.tile([C, NCHUNK], f32, tag="o", name="bc")
                    for hh in range(nparts):
                        hs = slice(HC * hh, HC * (hh + 1))
                        nc.tensor.matmul(bc[:, hs], ones64[:, :],
                                         recs[hh][:, :],
                                         start=True, stop=True)
                    for hh in range(nparts):
                        hs = slice(HC * hh, HC * (hh + 1))
                        gs = slice(NCHUNK * t + HC * hh,
                                   NCHUNK * t + HC * (hh + 1))
                        t1 = fp.tile([C, HC], f32, tag=f"t1{hh}", name="t1")
                        nc.vector.tensor_mul(t1[:, :], oas[0:C, hs],
                                             bc[:, hs])
                        fin = fp.tile([C, HC], f32, tag=f"fin{hh}", name="fin")
                        nc.vector.tensor_add(fin[:, :], t1[:, :], xr_t[:, gs])
                        nc.sync.dma_start(out=out[:, gs], in_=fin[:, :])

            # ---- emission: energies per exp-group (3 m-blocks, distinct
            # row groups -> concurrent burst); exp(g) after its energies;
            # AV lags one group so the PE can run ahead of the ACT.
            kq_done = 0
            vt_done = 0

            for t in range(NT):
                oa = ops.tile([C + 1, NCHUNK], f32, tag="o", name="oa")
                exs = {}
                for g in range(NGRP):
                    jlast = min(GRP * (g + 1), NJ) - 1
                    if t == 0:
                        while kq_done <= min(jlast // 4, 2 * NT - 1):
                            emit_kq(kq_done)
                            kq_done += 1
                    for j in range(GRP * g, jlast + 1):
                        energy_mm(t, g, j, get_e(t * NGRP + g))
                    exs[g] = emit_exp(t, g)
                    if g >= 1:
                        if t == 0:
                            jprev = min(GRP * g, NJ) - 1
                            while vt_done <= min(jprev // 8, NJ // 8 - 1):
                                emit_vt(vt_done)
                                vt_done += 1
                        emit_av(oa, exs.pop(g - 1), g - 1)
                if t == 0:
                    while vt_done < NJ // 8:
                        emit_vt(vt_done)
                        vt_done += 1
                emit_av(oa, exs.pop(NGRP - 1), NGRP - 1)
                emit_epilogue(t, oa)

    nc.compile()
    return nc


def _get_compiled():
    if "nc" not in _compiled:
        _compiled["nc"] = _build()
    return _compiled["nc"]


def kernel(x, Wq, bq, Wk, bk, Wv, bv, gamma):
    global LAST_RESULT
    _ensure_ntff_hook_importable()
    from concourse.bass_utils import run_bass_kernel_spmd

    nc = _get_compiled()

    x = np.asarray(x, dtype=np.float32)
    xf = x.reshape(B, C, N)
    Wq, Wk, Wv = np.asarray(Wq), np.asarray(Wk), np.asarray(Wv)
    bq, bk, bv = np.asarray(bq), np.asarray(bk), np.asarray(bv)
    gval = float(np.asarray(gamma).reshape(-1)[0])

    # wqk [65, 16]: k weights at cols 0-7, q at cols 8-15, bias row at 64.
    wqk_a = np.zeros((C + 1, 16), np.float32)
    wqk_a[0:C, 0:INTER] = Wk.T
    wqk_a[C, 0:INTER] = bk
    wqk_a[0:C, INTER:2 * INTER] = Wq.T
    wqk_a[C, INTER:2 * INTER] = bq
    wqk_a = wqk_a.astype(ml_dtypes.bfloat16)
    wv_a = np.ascontiguousarray(gval * Wv.T).astype(ml_dtypes.bfloat16)

    in_maps = []
    for core in range(NCORES):
        b, h = divmod(core, 2)
        own = xf[b][:, h * NHALF:(h + 1) * NHALF]
        oth = xf[b][:, (1 - h) * NHALF:(2 - h) * NHALF]
        ones = np.ones((1, NHALF), dtype=np.float32)
        xbh_core = np.concatenate([own, ones, oth, ones],
                                  axis=0).astype(ml_dtypes.bfloat16)
        # gamma*bv folded into the residual (sum_m A[n,m] = denom[n])
        xres_core = own + gval * bv[:, None]
        in_maps.append({
            "xbh": np.ascontiguousarray(xbh_core),
            "xres": np.ascontiguousarray(xres_core, dtype=np.float32),
            "wqk": wqk_a, "wv_": wv_a,
        })

    trace = bool(os.environ.get("KTRACE"))
    res = run_bass_kernel_spmd(nc, in_maps, list(range(NCORES)), trace=trace)
    LAST_RESULT = res

    outf = np.empty((B, C, N), dtype=np.float32)
    for core in range(NCORES):
        b, h = divmod(core, 2)
        outf[b][:, h * NHALF:(h + 1) * NHALF] = res.results[core]["out"]
    return outf.reshape(B, C, H, W)
